# revision 24
# baseline (speedup 1.0000x reference)
"""Causal self-attention (B=4, T=2048, C=1024, H=16, hs=64) on 8 trn2 cores.

Sharding: core c = batch (c//2) x head-group (c%2, 8 heads each).
Each core computes, for its (batch, 8 heads):
  - QKV projection against its slice of w_attn (transposed layouts on chip),
  - causal softmax attention (flash-style, no max subtraction -- scores are
    O(1) for this problem so exp is numerically safe; softmax denominator
    comes for free as a 65th "ones" row appended to V in the PV matmul),
  - partial output projection against its 512 rows of w_o, transposed.
Host side: per-batch pair partials are summed (the tensor-parallel
all-reduce done at unshard time) and transposed back.

v2 schedule: the attention inner loop is ACT(exp)-paced; QKV- and
output-projection matmuls are emitted one at a time from a background
queue into the gaps so the PE stream never starves, and the two heads'
exps per key-chunk pair are merged into one FD=2048 activation.
"""

from collections import deque

import numpy as np
from ml_dtypes import bfloat16

import concourse.bass as bass
import concourse.mybir as mybir
import concourse.tile as tile

N_CORES = 8
B, T, C = 4, 2048, 1024
H_PER_CORE = 8
HS = 64
P = 128
KCH = C // P  # 8 contraction chunks for the projections
NCH = T // 512  # 4 token chunks of 512
QT = T // 512  # 4 query tiles of 512
NEG = -1.0e5
SCALE = 1.0 / np.sqrt(HS)

F32 = mybir.dt.float32
BF16 = mybir.dt.bfloat16


def _mm(nc, out, lhsT, rhs, **kw):
    """Matmul on bf16 operands (1 cycle/row; plain fp32 costs 4)."""
    nc.tensor.matmul(out, lhsT, rhs, **kw)


def legalize_sync_waits(nc, max_waits=1):
    """Split multi-sem-wait instructions into chains of single-wait nops.

    The installed walrus codegen rejects instructions whose sync_info
    carries more than one wait. Same-engine program order makes moving the
    extra waits onto preceding nops semantically identical.
    """
    for f in nc.m.functions:
        for b in f.blocks:
            new_insts = []
            for inst in b.instructions:
                si = inst.sync_info
                if si and si.on_wait and len(si.on_wait) > max_waits:
                    waits = list(si.on_wait)
                    extra, keep = waits[:-max_waits], waits[-max_waits:]
                    for i in range(0, len(extra), max_waits):
                        chunk = extra[i : i + max_waits]
                        nop = mybir.InstNoOp(
                            name=f"{inst.name}-ws{i}",
                            engine=inst.engine,
                            ins=[],
                            outs=[],
                            sync_info=mybir.SyncInfo(on_wait=chunk, on_update=[]),
                        )
                        new_insts.append(nop)
                    inst.sync_info = mybir.SyncInfo(
                        on_wait=keep, on_update=list(si.on_update or [])
                    )
                new_insts.append(inst)
            b.instructions = new_insts


def build_attention_body(nc, tc, ctx, tensors, opts=None):
    """Emit one iteration of the per-core attention computation.

    The attention loop is the spine; projection matmuls drip in from a
    background queue (3 per attention group) so the PE never idles while
    the ACT engine works through the exps that gate each PV step.
    """
    x_t, w_qk, w_v, w_o, mask, out_t = tensors
    opts = dict(opts or {})
    Exp = mybir.ActivationFunctionType.Exp
    BG_PER_GROUP = opts.get("bg_per_group", 4)

    x_t3 = x_t[:].rearrange("(ko ki) t -> ki ko t", ki=P)  # [128, 8, 2048]
    w_qk3 = w_qk[:].rearrange("(ko ki) m -> ki ko m", ki=P)  # [128, 8, 1024]
    w_v3 = w_v[:].rearrange("(ko ki) m -> ki ko m", ki=P)  # [128, 8, 512]
    w_o3 = w_o[:].rearrange("(ko ki) m -> ki ko m", ki=P)  # [128, 4, 1024]
    out2 = out_t[:]  # [1024, 2048]

    consts = ctx.enter_context(tc.tile_pool(name="consts", bufs=1))
    big = ctx.enter_context(tc.tile_pool(name="big", bufs=1))
    expp = ctx.enter_context(tc.tile_pool(name="expp", bufs=opts.get("expp_bufs", 3)))
    smallp = ctx.enter_context(tc.tile_pool(name="smallp", bufs=2))
    outsb = ctx.enter_context(tc.tile_pool(name="outsb", bufs=2))
    psum = ctx.enter_context(tc.tile_pool(name="psum", bufs=2, space="PSUM"))

    mask2_sb = consts.tile([P, 2, P], F32, name="mask2_sb")
    ones_sb = consts.tile([1, HS], BF16, name="ones_sb")
    nc.vector.memset(ones_sb, 1.0)
    w_v_sb = consts.tile([P, KCH, 512], BF16, name="w_v_sb")
    w_o_sb = consts.tile([P, 4, 1024], BF16, name="w_o_sb")

    # Resident inputs and per head-pair Q^T / K^T / Y^T tiles.
    xt_sb = big.tile([P, KCH, T], BF16, name="xt_sb")
    wqk_sb = big.tile([P, KCH, 1024], BF16, name="wqk_sb")
    q_sb = [big.tile([P, T], BF16, name=f"q_sb{p}") for p in range(4)]
    k_sb = [big.tile([P, T], BF16, name=f"k_sb{p}") for p in range(4)]
    y_sb = [big.tile([P, T], BF16, name=f"y_sb{p}") for p in range(4)]
    # V' = [V | 1] per head: [key%128, keychunk, head, 65]
    vp_sb = big.tile([P, T // P, H_PER_CORE, HS + 1], BF16, name="vp_sb")
    nc.vector.memset(vp_sb[:, :, :, HS : HS + 1], 1.0)

    # Input DMAs, in first-use order: p1(0) block mt only needs x chunk 0
    # plus its own 256KB wqk slice, so interleave those to cut the startup
    # stall; w_v is needed mid-p1(0); late x chunks and w_o trail.
    nc.sync.dma_start(
        out=wqk_sb[:, :, 0:P], in_=w_qk3[:, :, 0:P]
    )
    nc.sync.dma_start(out=xt_sb[:, :, 0:512], in_=x_t3[:, :, 0:512])
    for i in (0, 1):
        nc.sync.dma_start(out=mask2_sb[:, i, :], in_=mask[:])
    for mt in range(1, 8):
        nc.sync.dma_start(
            out=wqk_sb[:, :, mt * P : (mt + 1) * P],
            in_=w_qk3[:, :, mt * P : (mt + 1) * P],
        )
    nc.sync.dma_start(out=w_v_sb, in_=w_v3)
    for nch in range(1, NCH):
        ts0 = nch * 512
        nc.sync.dma_start(
            out=xt_sb[:, :, ts0 : ts0 + 512], in_=x_t3[:, :, ts0 : ts0 + 512]
        )
    nc.sync.dma_start(out=w_o_sb, in_=w_o3)

    def evict(out, in_):
        nc.vector.tensor_copy(out=out, in_=in_)

    # ---- background work: one yield ~= one PE matmul ----

    def p1_qk_block(nch, mt):
        ts0 = nch * 512
        ps = psum.tile([P, 512], F32, name=f"p1_{nch}_{mt}", tag="acc", bufs=2)
        for kc in range(KCH):
            _mm(
                nc,
                ps,
                wqk_sb[:, kc, mt * P : (mt + 1) * P],
                xt_sb[:, kc, ts0 : ts0 + 512],
                start=(kc == 0),
                stop=(kc == KCH - 1),
            )
            yield
        dst = q_sb[mt] if mt < 4 else k_sb[mt - 4]
        evict(dst[:, ts0 : ts0 + 512], ps)
        yield

    def p1_v_block(nch, tt):
        ts0 = nch * 512
        ps = psum.tile([P, 512], F32, name=f"pv_{nch}_{tt}", tag="acc", bufs=2)
        for kc in range(KCH):
            _mm(
                nc,
                ps,
                xt_sb[:, kc, ts0 + tt * P : ts0 + (tt + 1) * P],
                w_v_sb[:, kc, :],
                start=(kc == 0),
                stop=(kc == KCH - 1),
            )
            yield
        evict(
            vp_sb[:, nch * 4 + tt, :, 0:HS],
            ps.rearrange("p (h d) -> p h d", h=H_PER_CORE),
        )
        yield

    def po_block(nch, mt):
        ts0 = nch * 512
        ps = psum.tile([P, 512], F32, name=f"po_{nch}_{mt}", tag="acc", bufs=2)
        for kc in range(4):
            _mm(
                nc,
                ps,
                w_o_sb[:, kc, mt * P : (mt + 1) * P],
                y_sb[kc][:, ts0 : ts0 + 512],
                start=(kc == 0),
                stop=(kc == 3),
            )
            yield
        ot = outsb.tile([P, 512], F32, name=f"ot_{nch}_{mt}", tag="ot")
        evict(ot, ps)
        nc.sync.dma_start(
            out=out2[mt * P : (mt + 1) * P, ts0 : ts0 + 512], in_=ot
        )
        yield

    bg = deque()  # (key, generator); key identifies the block for gating

    def enqueue_p1(nch):
        # Order matches first use by the attention spine: pair p's attention
        # needs q_p at its start and k_p plus V at its diagonal key chunks.
        bg.append((("p1", nch, "q0"), p1_qk_block(nch, 0)))
        bg.append((("p1", nch, "k0"), p1_qk_block(nch, 4)))
        for tt in range(4):
            bg.append((("p1", nch, f"v{tt}"), p1_v_block(nch, tt)))
        for p in range(1, 4):
            bg.append((("p1", nch, f"q{p}"), p1_qk_block(nch, p)))
            bg.append((("p1", nch, f"k{p}"), p1_qk_block(nch, 4 + p)))

    def enqueue_po(nch):
        for mt in range(8):
            bg.append((("po", nch, mt), po_block(nch, mt)))

    def drain_bg(n):
        done = 0
        while bg and done < n:
            try:
                next(bg[0][1])
                done += 1
            except StopIteration:
                bg.popleft()

    def drain_through(key):
        """Exhaust queued generators from the head through `key` (FIFO order
        means everything enqueued before `key` is drained too)."""
        while any(k == key for k, _ in bg):
            try:
                next(bg[0][1])
            except StopIteration:
                bg.popleft()

    # ---- the attention spine ----

    def attn_pair(qt, p, kc2):
        """Two key chunks batched: 4 row-tiled S matmuls (one 64-row mode
        span), the 2-head exp per chunk, then 4 PV matmuls (one 128-row
        span). Halves the PE tiling-mode switches vs per-chunk emission."""
        q0 = qt * 512
        nkc = 4 * (qt + 1)
        sws, ews, qoffs = [], [], []
        for ci in (0, 1):
            kc = kc2 + ci
            c = kc - 4 * qt
            qoff = max(0, c) * P
            qoffs.append(qoff)
            if c == 0:
                drain_through(("p1", qt, f"k{p}"))
            sw = psum.tile(
                [P, 2, 512], F32, name=f"sw_{qt}_{p}_{kc}", tag="sw", bufs=2
            )
            sws.append(sw)
            for i in (0, 1):  # heads 2p / 2p+1 at array rows 0-63 / 64-127
                r0 = i * HS
                _mm(
                    nc,
                    sw[:, i, qoff:],
                    k_sb[p][r0 : r0 + HS, kc * P : (kc + 1) * P],
                    q_sb[p][r0 : r0 + HS, q0 + qoff : q0 + 512],
                    start=True,
                    stop=True,
                )
        for ci in (0, 1):
            kc = kc2 + ci
            c = kc - 4 * qt
            qoff = qoffs[ci]
            if c >= 0:
                nc.vector.tensor_add(
                    out=sws[ci][:, :, qoff : qoff + P],
                    in0=sws[ci][:, :, qoff : qoff + P],
                    in1=mask2_sb,
                )
            ew = expp.tile([P, 2, 512], BF16, name=f"e_{qt}_{p}_{kc}", tag="exps")
            ews.append(ew)
            nc.scalar.activation(
                out=ew[:, :, qoff:], in_=sws[ci][:, :, qoff:], func=Exp, scale=SCALE
            )
            if c >= 0:
                drain_through(("p1", qt, f"v{c}"))  # PV needs vp chunk kc
        drain_bg(BG_PER_GROUP)
        for ci in (0, 1):
            kc = kc2 + ci
            qoff = qoffs[ci]
            for i in (0, 1):
                _mm(
                    nc,
                    o_ab[i][:, qoff:],
                    vp_sb[:, kc, 2 * p + i, :],
                    ews[ci][:, i, qoff:],
                    start=(kc == 0),
                    stop=(kc == nkc - 1),
                )

    def attn_tail(qt, p, i, o_ps):
        """1/l from the ones row; broadcast via a K=1 matmul; scale into y."""
        q0 = qt * 512
        h = 2 * p + i
        pp, r0 = h // 2, (h % 2) * HS
        recip = smallp.tile([1, 512], BF16, name=f"r_{qt}_{h}", tag="recip")
        with nc.allow_low_precision(reason="1/l in bf16 for the broadcast"):
            nc.vector.reciprocal(out=recip, in_=o_ps[HS : HS + 1, :])
        bc_ps = psum.tile([P, 512], F32, name=f"bc_{qt}_{h}", tag="acc", bufs=2)
        _mm(nc, bc_ps[0:HS, :], ones_sb, recip, start=True, stop=True)
        bc_sb = smallp.tile([HS, 512], BF16, name=f"bs_{qt}_{h}", tag="bcs")
        nc.vector.tensor_copy(out=bc_sb, in_=bc_ps[0:HS, :])
        nc.vector.tensor_mul(
            out=y_sb[pp][r0 : r0 + HS, q0 : q0 + 512],
            in0=o_ps[0:HS, :],
            in1=bc_sb,
        )

    for nch in range(NCH):
        enqueue_p1(nch)
    for qt in range(QT):
        nkc = 4 * (qt + 1)
        for p in range(4):
            drain_through(("p1", qt, f"q{p}"))
            o_ab = [
                psum.tile(
                    [HS + 1, 512], F32, name=f"o_{qt}_{p}_{i}", tag="o", bufs=2
                )
                for i in (0, 1)
            ]
            for kc2 in range(0, nkc, 2):
                attn_pair(qt, p, kc2)
            for i in (0, 1):
                attn_tail(qt, p, i, o_ab[i])
                drain_bg(2)
        enqueue_po(qt)
    drain_bg(1 << 30)


def build_nc(loop_k=None, opts=None):
    """Build the per-core Bass module. loop_k wraps the body in a timing loop."""
    from contextlib import ExitStack

    nc = bass.Bass("TRN2")
    x_t = nc.dram_tensor("x_t", [C, T], BF16, kind="ExternalInput")
    w_qk = nc.dram_tensor("w_qk", [C, 1024], BF16, kind="ExternalInput")
    w_v = nc.dram_tensor("w_v", [C, 512], BF16, kind="ExternalInput")
    w_o = nc.dram_tensor("w_o", [512, C], BF16, kind="ExternalInput")
    mask = nc.dram_tensor("mask", [P, P], F32, kind="ExternalInput")
    out_t = nc.dram_tensor("out_t", [C, T], F32, kind="ExternalOutput")
    tensors = (x_t, w_qk, w_v, w_o, mask, out_t)

    with tile.TileContext(nc) as tc:
        with ExitStack() as ctx:
            if loop_k is None:
                build_attention_body(nc, tc, ctx, tensors, opts)
            else:
                with tc.For_i(0, loop_k, 1):
                    build_attention_body(nc, tc, ctx, tensors, opts)
    legalize_sync_waits(nc)
    return nc


def shard_inputs(x, w_attn, w_o):
    """Build the 8 per-core input maps."""
    x = np.asarray(x, dtype=np.float32)
    w_attn = np.asarray(w_attn, dtype=np.float32)
    w_o = np.asarray(w_o, dtype=np.float32)
    w_q, w_k, w_v = w_attn[:, 0:C], w_attn[:, C : 2 * C], w_attn[:, 2 * C : 3 * C]
    r = np.arange(P)
    mask = np.where(r[:, None] > r[None, :], np.float32(NEG), np.float32(0.0))
    in_maps = []
    for c in range(N_CORES):
        b, g = c // 2, c % 2
        heads = range(g * H_PER_CORE, (g + 1) * H_PER_CORE)
        w_qk_c = np.concatenate(
            [w_q[:, h * HS : (h + 1) * HS] for h in heads]
            + [w_k[:, h * HS : (h + 1) * HS] for h in heads],
            axis=1,
        )
        w_v_c = np.concatenate([w_v[:, h * HS : (h + 1) * HS] for h in heads], axis=1)
        in_maps.append(
            {
                "x_t": np.ascontiguousarray(x[b].T).astype(bfloat16),
                "w_qk": np.ascontiguousarray(w_qk_c).astype(bfloat16),
                "w_v": np.ascontiguousarray(w_v_c).astype(bfloat16),
                "w_o": np.ascontiguousarray(
                    w_o[g * 512 : (g + 1) * 512, :]
                ).astype(bfloat16),
                "mask": np.ascontiguousarray(mask, dtype=np.float32),
            }
        )
    return in_maps


def unshard_output(results):
    """Sum per-batch pair partials (the TP all-reduce) and untranspose."""
    out = np.empty((B, T, C), dtype=np.float32)
    for b in range(B):
        acc = results[2 * b]["out_t"] + results[2 * b + 1]["out_t"]
        out[b] = acc.T
    return out


# ---------------------------------------------------------------------------
# PJRT SPMD execution (axon): jit a shard_map over the 8 cores.
# ---------------------------------------------------------------------------


class SpmdRunner:
    def __init__(self, nc, n_cores=N_CORES):
        import jax
        from jax.sharding import Mesh, PartitionSpec
        from jax.experimental.shard_map import shard_map
        from concourse.bass2jax import (
            _bass_exec_p,
            install_neuronx_cc_hook,
            partition_id_tensor,
        )

        install_neuronx_cc_hook()
        self.jax = jax
        self.n_cores = n_cores
        partition_name = nc.partition_id_tensor.name if nc.partition_id_tensor else None
        in_names, out_names, out_avals, zero_outs = [], [], [], []
        for alloc in nc.m.functions[0].allocations:
            if not isinstance(alloc, mybir.MemoryLocationSet):
                continue
            name = alloc.memorylocations[0].name
            if alloc.kind == "ExternalInput":
                if name != partition_name:
                    in_names.append(name)
            elif alloc.kind == "ExternalOutput":
                out_names.append(name)
                shape = tuple(alloc.tensor_shape)
                dtype = mybir.dt.np(alloc.dtype)
                out_avals.append(jax.core.ShapedArray(shape, dtype))
                zero_outs.append(np.zeros(shape, dtype))
        self.in_names, self.out_names = in_names, out_names
        self.out_avals, self.zero_outs = out_avals, zero_outs
        n_params, n_outs = len(in_names), len(out_avals)
        all_in_names = in_names + out_names
        if partition_name is not None:
            all_in_names.append(partition_name)

        def _body(*args):
            operands = list(args)
            if partition_name is not None:
                operands.append(partition_id_tensor())
            return tuple(
                _bass_exec_p.bind(
                    *operands,
                    out_avals=tuple(out_avals),
                    in_names=tuple(all_in_names),
                    out_names=tuple(out_names),
                    lowering_input_output_aliases=(),
                    sim_require_finite=True,
                    sim_require_nnan=True,
                    nc=nc,
                )
            )

        devices = jax.devices()[:n_cores]
        assert len(devices) == n_cores, f"need {n_cores} cores, saw {jax.devices()}"
        self.mesh = Mesh(np.asarray(devices), ("core",))
        self.pspec = PartitionSpec("core")
        in_specs = (self.pspec,) * (n_params + n_outs)
        out_specs = (self.pspec,) * len(out_names)
        self.sharded = jax.jit(
            shard_map(
                _body,
                mesh=self.mesh,
                in_specs=in_specs,
                out_specs=out_specs,
                check_rep=False,
            ),
            keep_unused=True,
        )
        self.n_params = n_params

    def prepare(self, in_maps):
        from jax.sharding import NamedSharding

        per_core = [[np.asarray(m[n]) for n in self.in_names] for m in in_maps]
        concat_in = [
            np.concatenate([per_core[c][i] for c in range(self.n_cores)], axis=0)
            for i in range(self.n_params)
        ]
        concat_zeros = [
            np.zeros((self.n_cores * z.shape[0], *z.shape[1:]), z.dtype)
            for z in self.zero_outs
        ]
        sharding = NamedSharding(self.mesh, self.pspec)
        args = [self.jax.device_put(a, sharding) for a in concat_in + concat_zeros]
        self.jax.block_until_ready(args)
        return args

    def run(self, args):
        out = self.sharded(*args)
        self.jax.block_until_ready(out)
        return out

    def results(self, out_arrs):
        return [
            {
                name: np.asarray(out_arrs[i]).reshape(
                    self.n_cores, *self.out_avals[i].shape
                )[c]
                for i, name in enumerate(self.out_names)
            }
            for c in range(self.n_cores)
        ]


_RUNNER = None


def _get_runner():
    global _RUNNER
    if _RUNNER is None:
        _RUNNER = SpmdRunner(build_nc())
    return _RUNNER


def kernel(x, w_attn, w_o):
    runner = _get_runner()
    in_maps = shard_inputs(x, w_attn, w_o)
    args = runner.prepare(in_maps)
    out = runner.run(args)
    return unshard_output(runner.results(out))


# revision 26
# speedup vs baseline: 1.0470x; 1.0470x over previous
"""Causal self-attention (B=4, T=2048, C=1024, H=16, hs=64) on 8 trn2 cores.

Sharding: core c = batch (c//2) x head-group (c%2, 8 heads each).
Each core computes, for its (batch, 8 heads):
  - QKV projection against its slice of w_attn (transposed layouts on chip),
  - causal softmax attention (flash-style, no max subtraction -- scores are
    O(1) for this problem so exp is numerically safe; softmax denominator
    comes for free as a 65th "ones" row appended to V in the PV matmul),
  - partial output projection against its 512 rows of w_o, transposed.
Host side: per-batch pair partials are summed (the tensor-parallel
all-reduce done at unshard time) and transposed back.

v2 schedule: the attention inner loop is ACT(exp)-paced; QKV- and
output-projection matmuls are emitted one at a time from a background
queue into the gaps so the PE stream never starves, and the two heads'
exps per key-chunk pair are merged into one FD=2048 activation.
"""

from collections import deque

import numpy as np
from ml_dtypes import bfloat16

import concourse.bass as bass
import concourse.mybir as mybir
import concourse.tile as tile

N_CORES = 8
B, T, C = 4, 2048, 1024
H_PER_CORE = 8
HS = 64
P = 128
KCH = C // P  # 8 contraction chunks for the projections
NCH = T // 512  # 4 token chunks of 512
QT = T // 512  # 4 query tiles of 512
NEG = -1.0e5
SCALE = 1.0 / np.sqrt(HS)

F32 = mybir.dt.float32
BF16 = mybir.dt.bfloat16


def _mm(nc, out, lhsT, rhs, **kw):
    """Matmul on bf16 operands (1 cycle/row; plain fp32 costs 4)."""
    nc.tensor.matmul(out, lhsT, rhs, **kw)


def legalize_sync_waits(nc, max_waits=1):
    """Split multi-sem-wait instructions into chains of single-wait nops.

    The installed walrus codegen rejects instructions whose sync_info
    carries more than one wait. Same-engine program order makes moving the
    extra waits onto preceding nops semantically identical.
    """
    for f in nc.m.functions:
        for b in f.blocks:
            new_insts = []
            for inst in b.instructions:
                si = inst.sync_info
                if si and si.on_wait and len(si.on_wait) > max_waits:
                    waits = list(si.on_wait)
                    extra, keep = waits[:-max_waits], waits[-max_waits:]
                    for i in range(0, len(extra), max_waits):
                        chunk = extra[i : i + max_waits]
                        nop = mybir.InstNoOp(
                            name=f"{inst.name}-ws{i}",
                            engine=inst.engine,
                            ins=[],
                            outs=[],
                            sync_info=mybir.SyncInfo(on_wait=chunk, on_update=[]),
                        )
                        new_insts.append(nop)
                    inst.sync_info = mybir.SyncInfo(
                        on_wait=keep, on_update=list(si.on_update or [])
                    )
                new_insts.append(inst)
            b.instructions = new_insts


def build_attention_body(nc, tc, ctx, tensors, opts=None):
    """Emit one iteration of the per-core attention computation.

    The attention loop is the spine; projection matmuls drip in from a
    background queue (3 per attention group) so the PE never idles while
    the ACT engine works through the exps that gate each PV step.
    """
    x_t, w_qk, w_v, w_o, mask, out_t = tensors
    opts = dict(opts or {})
    Exp = mybir.ActivationFunctionType.Exp
    BG_PER_GROUP = opts.get("bg_per_group", 3)

    x_t3 = x_t[:].rearrange("(ko ki) t -> ki ko t", ki=P)  # [128, 8, 2048]
    w_qk3 = w_qk[:].rearrange("(ko ki) m -> ki ko m", ki=P)  # [128, 8, 1024]
    w_v3 = w_v[:].rearrange("(ko ki) m -> ki ko m", ki=P)  # [128, 8, 512]
    w_o3 = w_o[:].rearrange("(ko ki) m -> ki ko m", ki=P)  # [128, 4, 1024]
    out2 = out_t[:]  # [1024, 2048]

    consts = ctx.enter_context(tc.tile_pool(name="consts", bufs=1))
    big = ctx.enter_context(tc.tile_pool(name="big", bufs=1))
    expp = ctx.enter_context(tc.tile_pool(name="expp", bufs=opts.get("expp_bufs", 3)))
    smallp = ctx.enter_context(tc.tile_pool(name="smallp", bufs=2))
    outsb = ctx.enter_context(tc.tile_pool(name="outsb", bufs=2))
    psum = ctx.enter_context(tc.tile_pool(name="psum", bufs=2, space="PSUM"))

    mask2_sb = consts.tile([P, 2, P], F32, name="mask2_sb")
    ones_sb = consts.tile([1, HS], BF16, name="ones_sb")
    nc.vector.memset(ones_sb, 1.0)
    w_v_sb = consts.tile([P, KCH, 512], BF16, name="w_v_sb")
    w_o_sb = consts.tile([P, 4, 1024], BF16, name="w_o_sb")

    # Resident inputs and per head-pair Q^T / K^T / Y^T tiles.
    xt_sb = big.tile([P, KCH, T], BF16, name="xt_sb")
    wqk_sb = big.tile([P, KCH, 1024], BF16, name="wqk_sb")
    q_sb = [big.tile([P, T], BF16, name=f"q_sb{p}") for p in range(4)]
    k_sb = [big.tile([P, T], BF16, name=f"k_sb{p}") for p in range(4)]
    y_sb = [big.tile([P, T], BF16, name=f"y_sb{p}") for p in range(4)]
    # V' = [V | 1] per head: [key%128, keychunk, head, 65]
    vp_sb = big.tile([P, T // P, H_PER_CORE, HS + 1], BF16, name="vp_sb")
    nc.vector.memset(vp_sb[:, :, :, HS : HS + 1], 1.0)

    # Input DMAs, in first-use order: p1(0) block mt only needs x chunk 0
    # plus its own 256KB wqk slice, so interleave those to cut the startup
    # stall; w_v is needed mid-p1(0); late x chunks and w_o trail.
    nc.sync.dma_start(
        out=wqk_sb[:, :, 0:P], in_=w_qk3[:, :, 0:P]
    )
    nc.sync.dma_start(out=xt_sb[:, :, 0:512], in_=x_t3[:, :, 0:512])
    for i in (0, 1):
        nc.sync.dma_start(out=mask2_sb[:, i, :], in_=mask[:])
    for mt in range(1, 8):
        nc.sync.dma_start(
            out=wqk_sb[:, :, mt * P : (mt + 1) * P],
            in_=w_qk3[:, :, mt * P : (mt + 1) * P],
        )
    nc.sync.dma_start(out=w_v_sb, in_=w_v3)
    for nch in range(1, NCH):
        ts0 = nch * 512
        nc.sync.dma_start(
            out=xt_sb[:, :, ts0 : ts0 + 512], in_=x_t3[:, :, ts0 : ts0 + 512]
        )
    nc.sync.dma_start(out=w_o_sb, in_=w_o3)

    def evict(out, in_):
        nc.vector.tensor_copy(out=out, in_=in_)

    # ---- background work: one yield ~= one PE matmul ----

    def p1_qk_block(nch, mt):
        ts0 = nch * 512
        ps = psum.tile([P, 512], F32, name=f"p1_{nch}_{mt}", tag="acc", bufs=2)
        for kc in range(KCH):
            _mm(
                nc,
                ps,
                wqk_sb[:, kc, mt * P : (mt + 1) * P],
                xt_sb[:, kc, ts0 : ts0 + 512],
                start=(kc == 0),
                stop=(kc == KCH - 1),
            )
            yield
        dst = q_sb[mt] if mt < 4 else k_sb[mt - 4]
        evict(dst[:, ts0 : ts0 + 512], ps)
        yield

    def p1_v_block(nch, tt):
        ts0 = nch * 512
        ps = psum.tile([P, 512], F32, name=f"pv_{nch}_{tt}", tag="acc", bufs=2)
        for kc in range(KCH):
            _mm(
                nc,
                ps,
                xt_sb[:, kc, ts0 + tt * P : ts0 + (tt + 1) * P],
                w_v_sb[:, kc, :],
                start=(kc == 0),
                stop=(kc == KCH - 1),
            )
            yield
        evict(
            vp_sb[:, nch * 4 + tt, :, 0:HS],
            ps.rearrange("p (h d) -> p h d", h=H_PER_CORE),
        )
        yield

    def po_block(nch, mt):
        ts0 = nch * 512
        ps = psum.tile([P, 512], F32, name=f"po_{nch}_{mt}", tag="acc", bufs=2)
        for kc in range(4):
            _mm(
                nc,
                ps,
                w_o_sb[:, kc, mt * P : (mt + 1) * P],
                y_sb[kc][:, ts0 : ts0 + 512],
                start=(kc == 0),
                stop=(kc == 3),
            )
            yield
        ot = outsb.tile([P, 512], BF16, name=f"ot_{nch}_{mt}", tag="ot")
        evict(ot, ps)
        nc.sync.dma_start(
            out=out2[mt * P : (mt + 1) * P, ts0 : ts0 + 512], in_=ot
        )
        yield

    bg = deque()  # (key, generator); key identifies the block for gating

    def enqueue_p1(nch):
        # Order matches first use by the attention spine: pair p's attention
        # needs q_p at its start and k_p plus V at its diagonal key chunks.
        bg.append((("p1", nch, "q0"), p1_qk_block(nch, 0)))
        bg.append((("p1", nch, "k0"), p1_qk_block(nch, 4)))
        for tt in range(4):
            bg.append((("p1", nch, f"v{tt}"), p1_v_block(nch, tt)))
        for p in range(1, 4):
            bg.append((("p1", nch, f"q{p}"), p1_qk_block(nch, p)))
            bg.append((("p1", nch, f"k{p}"), p1_qk_block(nch, 4 + p)))

    def enqueue_po(nch):
        for mt in range(8):
            bg.append((("po", nch, mt), po_block(nch, mt)))

    def drain_bg(n):
        done = 0
        while bg and done < n:
            try:
                next(bg[0][1])
                done += 1
            except StopIteration:
                bg.popleft()

    def drain_through(key):
        """Exhaust queued generators from the head through `key` (FIFO order
        means everything enqueued before `key` is drained too)."""
        while any(k == key for k, _ in bg):
            try:
                next(bg[0][1])
            except StopIteration:
                bg.popleft()

    # ---- the attention spine ----

    def attn_group(qt, p, kc):
        """S pair (row-tiled heads) -> one 2-head exp -> 2 PV matmuls."""
        q0 = qt * 512
        nkc = 4 * (qt + 1)
        c = kc - 4 * qt
        qoff = max(0, c) * P
        if c == 0:
            drain_through(("p1", qt, f"k{p}"))
        sw = psum.tile([P, 2, 512], F32, name=f"sw_{qt}_{p}_{kc}", tag="sw", bufs=2)
        for i in (0, 1):  # heads 2p / 2p+1 at array rows 0-63 / 64-127
            r0 = i * HS
            _mm(
                nc,
                sw[:, i, qoff:],
                k_sb[p][r0 : r0 + HS, kc * P : (kc + 1) * P],
                q_sb[p][r0 : r0 + HS, q0 + qoff : q0 + 512],
                start=True,
                stop=True,
            )
        if c >= 0:
            nc.vector.tensor_add(
                out=sw[:, :, qoff : qoff + P],
                in0=sw[:, :, qoff : qoff + P],
                in1=mask2_sb,
            )
        ew = expp.tile([P, 2, 512], BF16, name=f"e_{qt}_{p}_{kc}", tag="exps")
        nc.scalar.activation(
            out=ew[:, :, qoff:], in_=sw[:, :, qoff:], func=Exp, scale=SCALE
        )
        if c >= 0:
            drain_through(("p1", qt, f"v{c}"))  # PV below needs vp chunk kc
        drain_bg(BG_PER_GROUP)
        for i in (0, 1):
            _mm(
                nc,
                o_ab[i][:, qoff:],
                vp_sb[:, kc, 2 * p + i, :],
                ew[:, i, qoff:],
                start=(kc == 0),
                stop=(kc == nkc - 1),
            )

    def attn_tail(qt, p, i, o_ps):
        """1/l from the ones row; broadcast via a K=1 matmul; scale into y."""
        q0 = qt * 512
        h = 2 * p + i
        pp, r0 = h // 2, (h % 2) * HS
        recip = smallp.tile([1, 512], BF16, name=f"r_{qt}_{h}", tag="recip")
        with nc.allow_low_precision(reason="1/l in bf16 for the broadcast"):
            nc.vector.reciprocal(out=recip, in_=o_ps[HS : HS + 1, :])
        bc_ps = psum.tile([P, 512], F32, name=f"bc_{qt}_{h}", tag="acc", bufs=2)
        _mm(nc, bc_ps[0:HS, :], ones_sb, recip, start=True, stop=True)
        bc_sb = smallp.tile([HS, 512], BF16, name=f"bs_{qt}_{h}", tag="bcs")
        nc.vector.tensor_copy(out=bc_sb, in_=bc_ps[0:HS, :])
        nc.vector.tensor_mul(
            out=y_sb[pp][r0 : r0 + HS, q0 : q0 + 512],
            in0=o_ps[0:HS, :],
            in1=bc_sb,
        )

    for nch in range(NCH):
        enqueue_p1(nch)
    for qt in range(QT):
        nkc = 4 * (qt + 1)
        for p in range(4):
            drain_through(("p1", qt, f"q{p}"))
            o_ab = [
                psum.tile(
                    [HS + 1, 512], F32, name=f"o_{qt}_{p}_{i}", tag="o", bufs=2
                )
                for i in (0, 1)
            ]
            for kc in range(nkc):
                attn_group(qt, p, kc)
            for i in (0, 1):
                attn_tail(qt, p, i, o_ab[i])
                drain_bg(2)
        enqueue_po(qt)
    drain_bg(1 << 30)


def build_nc(loop_k=None, opts=None):
    """Build the per-core Bass module. loop_k wraps the body in a timing loop."""
    from contextlib import ExitStack

    nc = bass.Bass("TRN2")
    x_t = nc.dram_tensor("x_t", [C, T], BF16, kind="ExternalInput")
    w_qk = nc.dram_tensor("w_qk", [C, 1024], BF16, kind="ExternalInput")
    w_v = nc.dram_tensor("w_v", [C, 512], BF16, kind="ExternalInput")
    w_o = nc.dram_tensor("w_o", [512, C], BF16, kind="ExternalInput")
    mask = nc.dram_tensor("mask", [P, P], F32, kind="ExternalInput")
    out_t = nc.dram_tensor("out_t", [C, T], BF16, kind="ExternalOutput")
    tensors = (x_t, w_qk, w_v, w_o, mask, out_t)

    with tile.TileContext(nc) as tc:
        with ExitStack() as ctx:
            if loop_k is None:
                build_attention_body(nc, tc, ctx, tensors, opts)
            else:
                with tc.For_i(0, loop_k, 1):
                    build_attention_body(nc, tc, ctx, tensors, opts)
    legalize_sync_waits(nc)
    return nc


def shard_inputs(x, w_attn, w_o):
    """Build the 8 per-core input maps."""
    x = np.asarray(x, dtype=np.float32)
    w_attn = np.asarray(w_attn, dtype=np.float32)
    w_o = np.asarray(w_o, dtype=np.float32)
    w_q, w_k, w_v = w_attn[:, 0:C], w_attn[:, C : 2 * C], w_attn[:, 2 * C : 3 * C]
    r = np.arange(P)
    mask = np.where(r[:, None] > r[None, :], np.float32(NEG), np.float32(0.0))
    in_maps = []
    for c in range(N_CORES):
        b, g = c // 2, c % 2
        heads = range(g * H_PER_CORE, (g + 1) * H_PER_CORE)
        w_qk_c = np.concatenate(
            [w_q[:, h * HS : (h + 1) * HS] for h in heads]
            + [w_k[:, h * HS : (h + 1) * HS] for h in heads],
            axis=1,
        )
        w_v_c = np.concatenate([w_v[:, h * HS : (h + 1) * HS] for h in heads], axis=1)
        in_maps.append(
            {
                "x_t": np.ascontiguousarray(x[b].T).astype(bfloat16),
                "w_qk": np.ascontiguousarray(w_qk_c).astype(bfloat16),
                "w_v": np.ascontiguousarray(w_v_c).astype(bfloat16),
                "w_o": np.ascontiguousarray(
                    w_o[g * 512 : (g + 1) * 512, :]
                ).astype(bfloat16),
                "mask": np.ascontiguousarray(mask, dtype=np.float32),
            }
        )
    return in_maps


def unshard_output(results):
    """Sum per-batch pair partials (the TP all-reduce) and untranspose."""
    out = np.empty((B, T, C), dtype=np.float32)
    for b in range(B):
        acc = results[2 * b]["out_t"].astype(np.float32) + results[
            2 * b + 1
        ]["out_t"].astype(np.float32)
        out[b] = acc.T
    return out


# ---------------------------------------------------------------------------
# PJRT SPMD execution (axon): jit a shard_map over the 8 cores.
# ---------------------------------------------------------------------------


class SpmdRunner:
    def __init__(self, nc, n_cores=N_CORES):
        import jax
        from jax.sharding import Mesh, PartitionSpec
        from jax.experimental.shard_map import shard_map
        from concourse.bass2jax import (
            _bass_exec_p,
            install_neuronx_cc_hook,
            partition_id_tensor,
        )

        install_neuronx_cc_hook()
        self.jax = jax
        self.n_cores = n_cores
        partition_name = nc.partition_id_tensor.name if nc.partition_id_tensor else None
        in_names, out_names, out_avals, zero_outs = [], [], [], []
        for alloc in nc.m.functions[0].allocations:
            if not isinstance(alloc, mybir.MemoryLocationSet):
                continue
            name = alloc.memorylocations[0].name
            if alloc.kind == "ExternalInput":
                if name != partition_name:
                    in_names.append(name)
            elif alloc.kind == "ExternalOutput":
                out_names.append(name)
                shape = tuple(alloc.tensor_shape)
                dtype = mybir.dt.np(alloc.dtype)
                out_avals.append(jax.core.ShapedArray(shape, dtype))
                zero_outs.append(np.zeros(shape, dtype))
        self.in_names, self.out_names = in_names, out_names
        self.out_avals, self.zero_outs = out_avals, zero_outs
        n_params, n_outs = len(in_names), len(out_avals)
        all_in_names = in_names + out_names
        if partition_name is not None:
            all_in_names.append(partition_name)

        def _body(*args):
            operands = list(args)
            if partition_name is not None:
                operands.append(partition_id_tensor())
            return tuple(
                _bass_exec_p.bind(
                    *operands,
                    out_avals=tuple(out_avals),
                    in_names=tuple(all_in_names),
                    out_names=tuple(out_names),
                    lowering_input_output_aliases=(),
                    sim_require_finite=True,
                    sim_require_nnan=True,
                    nc=nc,
                )
            )

        devices = jax.devices()[:n_cores]
        assert len(devices) == n_cores, f"need {n_cores} cores, saw {jax.devices()}"
        self.mesh = Mesh(np.asarray(devices), ("core",))
        self.pspec = PartitionSpec("core")
        in_specs = (self.pspec,) * (n_params + n_outs)
        out_specs = (self.pspec,) * len(out_names)
        self.sharded = jax.jit(
            shard_map(
                _body,
                mesh=self.mesh,
                in_specs=in_specs,
                out_specs=out_specs,
                check_rep=False,
            ),
            keep_unused=True,
        )
        self.n_params = n_params

    def prepare(self, in_maps):
        from jax.sharding import NamedSharding

        per_core = [[np.asarray(m[n]) for n in self.in_names] for m in in_maps]
        concat_in = [
            np.concatenate([per_core[c][i] for c in range(self.n_cores)], axis=0)
            for i in range(self.n_params)
        ]
        concat_zeros = [
            np.zeros((self.n_cores * z.shape[0], *z.shape[1:]), z.dtype)
            for z in self.zero_outs
        ]
        sharding = NamedSharding(self.mesh, self.pspec)
        args = [self.jax.device_put(a, sharding) for a in concat_in + concat_zeros]
        self.jax.block_until_ready(args)
        return args

    def run(self, args):
        out = self.sharded(*args)
        self.jax.block_until_ready(out)
        return out

    def results(self, out_arrs):
        return [
            {
                name: np.asarray(out_arrs[i]).reshape(
                    self.n_cores, *self.out_avals[i].shape
                )[c]
                for i, name in enumerate(self.out_names)
            }
            for c in range(self.n_cores)
        ]


_RUNNER = None


def _get_runner():
    global _RUNNER
    if _RUNNER is None:
        _RUNNER = SpmdRunner(build_nc())
    return _RUNNER


def kernel(x, w_attn, w_o):
    runner = _get_runner()
    in_maps = shard_inputs(x, w_attn, w_o)
    args = runner.prepare(in_maps)
    out = runner.run(args)
    return unshard_output(runner.results(out))


# revision 28
# speedup vs baseline: 1.0813x; 1.0328x over previous
"""Causal self-attention (B=4, T=2048, C=1024, H=16, hs=64) on 8 trn2 cores.

Sharding: core c = batch (c//2) x head-group (c%2, 8 heads each).
Each core computes, for its (batch, 8 heads):
  - QKV projection against its slice of w_attn (transposed layouts on chip),
  - causal softmax attention (flash-style, no max subtraction -- scores are
    O(1) for this problem so exp is numerically safe; softmax denominator
    comes for free as a 65th "ones" row appended to V in the PV matmul),
  - partial output projection against its 512 rows of w_o, transposed.
Host side: per-batch pair partials are summed (the tensor-parallel
all-reduce done at unshard time) and transposed back.

v2 schedule: the attention inner loop is ACT(exp)-paced; QKV- and
output-projection matmuls are emitted one at a time from a background
queue into the gaps so the PE stream never starves, and the two heads'
exps per key-chunk pair are merged into one FD=2048 activation.
"""

from collections import deque

import numpy as np
from ml_dtypes import bfloat16

import concourse.bass as bass
import concourse.mybir as mybir
import concourse.tile as tile

N_CORES = 8
B, T, C = 4, 2048, 1024
H_PER_CORE = 8
HS = 64
P = 128
KCH = C // P  # 8 contraction chunks for the projections
NCH = T // 512  # 4 token chunks of 512
QT = T // 512  # 4 query tiles of 512
NEG = -1.0e5
SCALE = 1.0 / np.sqrt(HS)

F32 = mybir.dt.float32
BF16 = mybir.dt.bfloat16


def _mm(nc, out, lhsT, rhs, **kw):
    """Matmul on bf16 operands (1 cycle/row; plain fp32 costs 4)."""
    nc.tensor.matmul(out, lhsT, rhs, **kw)


def legalize_sync_waits(nc, max_waits=1):
    """Split multi-sem-wait instructions into chains of single-wait nops.

    The installed walrus codegen rejects instructions whose sync_info
    carries more than one wait. Same-engine program order makes moving the
    extra waits onto preceding nops semantically identical.
    """
    for f in nc.m.functions:
        for b in f.blocks:
            new_insts = []
            for inst in b.instructions:
                si = inst.sync_info
                if si and si.on_wait and len(si.on_wait) > max_waits:
                    waits = list(si.on_wait)
                    extra, keep = waits[:-max_waits], waits[-max_waits:]
                    for i in range(0, len(extra), max_waits):
                        chunk = extra[i : i + max_waits]
                        nop = mybir.InstNoOp(
                            name=f"{inst.name}-ws{i}",
                            engine=inst.engine,
                            ins=[],
                            outs=[],
                            sync_info=mybir.SyncInfo(on_wait=chunk, on_update=[]),
                        )
                        new_insts.append(nop)
                    inst.sync_info = mybir.SyncInfo(
                        on_wait=keep, on_update=list(si.on_update or [])
                    )
                new_insts.append(inst)
            b.instructions = new_insts


def build_attention_body(nc, tc, ctx, tensors, opts=None):
    """Emit one iteration of the per-core attention computation.

    The attention loop is the spine; projection matmuls drip in from a
    background queue (3 per attention group) so the PE never idles while
    the ACT engine works through the exps that gate each PV step.
    """
    x_t, w_qk, w_v, w_o, mask, out_t = tensors
    opts = dict(opts or {})
    Exp = mybir.ActivationFunctionType.Exp
    BG_PER_GROUP = opts.get("bg_per_group", 3)

    x_t3 = x_t[:].rearrange("(ko ki) t -> ki ko t", ki=P)  # [128, 8, 2048]
    w_qk3 = w_qk[:].rearrange("(ko ki) m -> ki ko m", ki=P)  # [128, 8, 1024]
    w_v3 = w_v[:].rearrange("(ko ki) m -> ki ko m", ki=P)  # [128, 8, 512]
    w_o3 = w_o[:].rearrange("(ko ki) m -> ki ko m", ki=P)  # [128, 4, 1024]
    out2 = out_t[:]  # [1024, 2048]

    consts = ctx.enter_context(tc.tile_pool(name="consts", bufs=1))
    big = ctx.enter_context(tc.tile_pool(name="big", bufs=1))
    expp = ctx.enter_context(tc.tile_pool(name="expp", bufs=opts.get("expp_bufs", 4)))
    smallp = ctx.enter_context(tc.tile_pool(name="smallp", bufs=3))
    outsb = ctx.enter_context(tc.tile_pool(name="outsb", bufs=2))
    psum = ctx.enter_context(tc.tile_pool(name="psum", bufs=2, space="PSUM"))

    mask2_sb = consts.tile([P, 2, P], F32, name="mask2_sb")
    ones_sb = consts.tile([1, HS], BF16, name="ones_sb")
    nc.vector.memset(ones_sb, 1.0)
    w_v_sb = consts.tile([P, KCH, 512], BF16, name="w_v_sb")
    w_o_sb = consts.tile([P, 4, 1024], BF16, name="w_o_sb")

    # Resident inputs and per head-pair Q^T / K^T / Y^T tiles.
    xt_sb = big.tile([P, KCH, T], BF16, name="xt_sb")
    wqk_sb = big.tile([P, KCH, 1024], BF16, name="wqk_sb")
    q_sb = [big.tile([P, T], BF16, name=f"q_sb{p}") for p in range(4)]
    k_sb = [big.tile([P, T], BF16, name=f"k_sb{p}") for p in range(4)]
    y_sb = [big.tile([P, T], BF16, name=f"y_sb{p}") for p in range(4)]
    # V' = [V | 1] per head: [key%128, keychunk, head, 65]
    vp_sb = big.tile([P, T // P, H_PER_CORE, HS + 1], BF16, name="vp_sb")
    nc.vector.memset(vp_sb[:, :, :, HS : HS + 1], 1.0)

    # Input DMAs, in first-use order: p1(0) block mt only needs x chunk 0
    # plus its own 256KB wqk slice, so interleave those to cut the startup
    # stall; w_v is needed mid-p1(0); late x chunks and w_o trail.
    nc.sync.dma_start(
        out=wqk_sb[:, :, 0:P], in_=w_qk3[:, :, 0:P]
    )
    nc.sync.dma_start(out=xt_sb[:, :, 0:512], in_=x_t3[:, :, 0:512])
    for i in (0, 1):
        nc.sync.dma_start(out=mask2_sb[:, i, :], in_=mask[:])
    for mt in range(1, 8):
        nc.sync.dma_start(
            out=wqk_sb[:, :, mt * P : (mt + 1) * P],
            in_=w_qk3[:, :, mt * P : (mt + 1) * P],
        )
    nc.sync.dma_start(out=w_v_sb, in_=w_v3)
    for nch in range(1, NCH):
        ts0 = nch * 512
        nc.sync.dma_start(
            out=xt_sb[:, :, ts0 : ts0 + 512], in_=x_t3[:, :, ts0 : ts0 + 512]
        )
    nc.sync.dma_start(out=w_o_sb, in_=w_o3)

    def evict(out, in_):
        nc.vector.tensor_copy(out=out, in_=in_)

    # ---- background work: one yield ~= one PE matmul ----

    def p1_qk_block(nch, mt):
        ts0 = nch * 512
        ps = psum.tile([P, 512], F32, name=f"p1_{nch}_{mt}", tag="acc", bufs=2)
        for kc in range(KCH):
            _mm(
                nc,
                ps,
                wqk_sb[:, kc, mt * P : (mt + 1) * P],
                xt_sb[:, kc, ts0 : ts0 + 512],
                start=(kc == 0),
                stop=(kc == KCH - 1),
            )
            yield
        dst = q_sb[mt] if mt < 4 else k_sb[mt - 4]
        evict(dst[:, ts0 : ts0 + 512], ps)
        yield

    def p1_v_block(nch, tt):
        ts0 = nch * 512
        ps = psum.tile([P, 512], F32, name=f"pv_{nch}_{tt}", tag="acc", bufs=2)
        for kc in range(KCH):
            _mm(
                nc,
                ps,
                xt_sb[:, kc, ts0 + tt * P : ts0 + (tt + 1) * P],
                w_v_sb[:, kc, :],
                start=(kc == 0),
                stop=(kc == KCH - 1),
            )
            yield
        evict(
            vp_sb[:, nch * 4 + tt, :, 0:HS],
            ps.rearrange("p (h d) -> p h d", h=H_PER_CORE),
        )
        yield

    def po_block(nch, mt):
        ts0 = nch * 512
        ps = psum.tile([P, 512], F32, name=f"po_{nch}_{mt}", tag="acc", bufs=2)
        for kc in range(4):
            _mm(
                nc,
                ps,
                w_o_sb[:, kc, mt * P : (mt + 1) * P],
                y_sb[kc][:, ts0 : ts0 + 512],
                start=(kc == 0),
                stop=(kc == 3),
            )
            yield
        ot = outsb.tile([P, 512], BF16, name=f"ot_{nch}_{mt}", tag="ot")
        evict(ot, ps)
        nc.sync.dma_start(
            out=out2[mt * P : (mt + 1) * P, ts0 : ts0 + 512], in_=ot
        )
        yield

    bg = deque()  # (key, generator); key identifies the block for gating

    def enqueue_p1(nch):
        # Order matches first use by the attention spine: pair p's attention
        # needs q_p at its start and k_p plus V at its diagonal key chunks.
        bg.append((("p1", nch, "q0"), p1_qk_block(nch, 0)))
        bg.append((("p1", nch, "k0"), p1_qk_block(nch, 4)))
        for tt in range(4):
            bg.append((("p1", nch, f"v{tt}"), p1_v_block(nch, tt)))
        for p in range(1, 4):
            bg.append((("p1", nch, f"q{p}"), p1_qk_block(nch, p)))
            bg.append((("p1", nch, f"k{p}"), p1_qk_block(nch, 4 + p)))

    def enqueue_po(nch):
        for mt in range(8):
            bg.append((("po", nch, mt), po_block(nch, mt)))

    def drain_bg(n):
        done = 0
        while bg and done < n:
            try:
                next(bg[0][1])
                done += 1
            except StopIteration:
                bg.popleft()

    def drain_through(key):
        """Exhaust queued generators from the head through `key` (FIFO order
        means everything enqueued before `key` is drained too)."""
        while any(k == key for k, _ in bg):
            try:
                next(bg[0][1])
            except StopIteration:
                bg.popleft()

    # ---- the attention spine ----

    def attn_s_exp(qt, p, kc):
        """S pair (row-tiled heads) -> mask -> one 2-head exp."""
        q0 = qt * 512
        c = kc - 4 * qt
        qoff = max(0, c) * P
        if c == 0:
            drain_through(("p1", qt, f"k{p}"))
        sw = psum.tile([P, 2, 512], F32, name=f"sw_{qt}_{p}_{kc}", tag="sw", bufs=2)
        for i in (0, 1):  # heads 2p / 2p+1 at array rows 0-63 / 64-127
            r0 = i * HS
            _mm(
                nc,
                sw[:, i, qoff:],
                k_sb[p][r0 : r0 + HS, kc * P : (kc + 1) * P],
                q_sb[p][r0 : r0 + HS, q0 + qoff : q0 + 512],
                start=True,
                stop=True,
            )
        if c >= 0:
            nc.vector.tensor_add(
                out=sw[:, :, qoff : qoff + P],
                in0=sw[:, :, qoff : qoff + P],
                in1=mask2_sb,
            )
        ew = expp.tile([P, 2, 512], BF16, name=f"e_{qt}_{p}_{kc}", tag="exps")
        nc.scalar.activation(
            out=ew[:, :, qoff:], in_=sw[:, :, qoff:], func=Exp, scale=SCALE
        )
        if c >= 0:
            drain_through(("p1", qt, f"v{c}"))  # PV will need vp chunk kc
        return ew, qoff

    def attn_pv(qt, p, kc, ew, qoff):
        nkc = 4 * (qt + 1)
        for i in (0, 1):
            _mm(
                nc,
                o_ab[i][:, qoff:],
                vp_sb[:, kc, 2 * p + i, :],
                ew[:, i, qoff:],
                start=(kc == 0),
                stop=(kc == nkc - 1),
            )

    def attn_tail(qt, p, i, o_ps):
        """1/l from the ones row; broadcast via a K=1 matmul; scale into y."""
        q0 = qt * 512
        h = 2 * p + i
        pp, r0 = h // 2, (h % 2) * HS
        recip = smallp.tile([1, 512], BF16, name=f"r_{qt}_{h}", tag="recip")
        with nc.allow_low_precision(reason="1/l in bf16 for the broadcast"):
            nc.vector.reciprocal(out=recip, in_=o_ps[HS : HS + 1, :])
        bc_ps = psum.tile([P, 512], F32, name=f"bc_{qt}_{h}", tag="acc", bufs=2)
        _mm(nc, bc_ps[0:HS, :], ones_sb, recip, start=True, stop=True)
        bc_sb = smallp.tile([HS, 512], BF16, name=f"bs_{qt}_{h}", tag="bcs")
        nc.vector.tensor_copy(out=bc_sb, in_=bc_ps[0:HS, :])
        nc.vector.tensor_mul(
            out=y_sb[pp][r0 : r0 + HS, q0 : q0 + 512],
            in0=o_ps[0:HS, :],
            in1=bc_sb,
        )

    for nch in range(NCH):
        enqueue_p1(nch)
    for qt in range(QT):
        nkc = 4 * (qt + 1)
        for p in range(4):
            drain_through(("p1", qt, f"q{p}"))
            o_ab = [
                psum.tile(
                    [HS + 1, 512], F32, name=f"o_{qt}_{p}_{i}", tag="o", bufs=2
                )
                for i in (0, 1)
            ]
            pend = None
            for kc in range(nkc):
                s_exp = attn_s_exp(qt, p, kc)
                drain_bg(BG_PER_GROUP)
                if pend is not None:
                    attn_pv(qt, p, kc - 1, *pend)
                pend = s_exp
            attn_pv(qt, p, nkc - 1, *pend)
            for i in (0, 1):
                attn_tail(qt, p, i, o_ab[i])
                drain_bg(2)
        enqueue_po(qt)
    drain_bg(1 << 30)


def build_nc(loop_k=None, opts=None):
    """Build the per-core Bass module. loop_k wraps the body in a timing loop."""
    from contextlib import ExitStack

    nc = bass.Bass("TRN2")
    x_t = nc.dram_tensor("x_t", [C, T], BF16, kind="ExternalInput")
    w_qk = nc.dram_tensor("w_qk", [C, 1024], BF16, kind="ExternalInput")
    w_v = nc.dram_tensor("w_v", [C, 512], BF16, kind="ExternalInput")
    w_o = nc.dram_tensor("w_o", [512, C], BF16, kind="ExternalInput")
    mask = nc.dram_tensor("mask", [P, P], F32, kind="ExternalInput")
    out_t = nc.dram_tensor("out_t", [C, T], BF16, kind="ExternalOutput")
    tensors = (x_t, w_qk, w_v, w_o, mask, out_t)

    with tile.TileContext(nc) as tc:
        with ExitStack() as ctx:
            if loop_k is None:
                build_attention_body(nc, tc, ctx, tensors, opts)
            else:
                with tc.For_i(0, loop_k, 1):
                    build_attention_body(nc, tc, ctx, tensors, opts)
    legalize_sync_waits(nc)
    return nc


def shard_inputs(x, w_attn, w_o):
    """Build the 8 per-core input maps."""
    x = np.asarray(x, dtype=np.float32)
    w_attn = np.asarray(w_attn, dtype=np.float32)
    w_o = np.asarray(w_o, dtype=np.float32)
    w_q, w_k, w_v = w_attn[:, 0:C], w_attn[:, C : 2 * C], w_attn[:, 2 * C : 3 * C]
    r = np.arange(P)
    mask = np.where(r[:, None] > r[None, :], np.float32(NEG), np.float32(0.0))
    in_maps = []
    for c in range(N_CORES):
        b, g = c // 2, c % 2
        heads = range(g * H_PER_CORE, (g + 1) * H_PER_CORE)
        w_qk_c = np.concatenate(
            [w_q[:, h * HS : (h + 1) * HS] for h in heads]
            + [w_k[:, h * HS : (h + 1) * HS] for h in heads],
            axis=1,
        )
        w_v_c = np.concatenate([w_v[:, h * HS : (h + 1) * HS] for h in heads], axis=1)
        in_maps.append(
            {
                "x_t": np.ascontiguousarray(x[b].T).astype(bfloat16),
                "w_qk": np.ascontiguousarray(w_qk_c).astype(bfloat16),
                "w_v": np.ascontiguousarray(w_v_c).astype(bfloat16),
                "w_o": np.ascontiguousarray(
                    w_o[g * 512 : (g + 1) * 512, :]
                ).astype(bfloat16),
                "mask": np.ascontiguousarray(mask, dtype=np.float32),
            }
        )
    return in_maps


def unshard_output(results):
    """Sum per-batch pair partials (the TP all-reduce) and untranspose."""
    out = np.empty((B, T, C), dtype=np.float32)
    for b in range(B):
        acc = results[2 * b]["out_t"].astype(np.float32) + results[
            2 * b + 1
        ]["out_t"].astype(np.float32)
        out[b] = acc.T
    return out


# ---------------------------------------------------------------------------
# PJRT SPMD execution (axon): jit a shard_map over the 8 cores.
# ---------------------------------------------------------------------------


class SpmdRunner:
    def __init__(self, nc, n_cores=N_CORES):
        import jax
        from jax.sharding import Mesh, PartitionSpec
        from jax.experimental.shard_map import shard_map
        from concourse.bass2jax import (
            _bass_exec_p,
            install_neuronx_cc_hook,
            partition_id_tensor,
        )

        install_neuronx_cc_hook()
        self.jax = jax
        self.n_cores = n_cores
        partition_name = nc.partition_id_tensor.name if nc.partition_id_tensor else None
        in_names, out_names, out_avals, zero_outs = [], [], [], []
        for alloc in nc.m.functions[0].allocations:
            if not isinstance(alloc, mybir.MemoryLocationSet):
                continue
            name = alloc.memorylocations[0].name
            if alloc.kind == "ExternalInput":
                if name != partition_name:
                    in_names.append(name)
            elif alloc.kind == "ExternalOutput":
                out_names.append(name)
                shape = tuple(alloc.tensor_shape)
                dtype = mybir.dt.np(alloc.dtype)
                out_avals.append(jax.core.ShapedArray(shape, dtype))
                zero_outs.append(np.zeros(shape, dtype))
        self.in_names, self.out_names = in_names, out_names
        self.out_avals, self.zero_outs = out_avals, zero_outs
        n_params, n_outs = len(in_names), len(out_avals)
        all_in_names = in_names + out_names
        if partition_name is not None:
            all_in_names.append(partition_name)

        def _body(*args):
            operands = list(args)
            if partition_name is not None:
                operands.append(partition_id_tensor())
            return tuple(
                _bass_exec_p.bind(
                    *operands,
                    out_avals=tuple(out_avals),
                    in_names=tuple(all_in_names),
                    out_names=tuple(out_names),
                    lowering_input_output_aliases=(),
                    sim_require_finite=True,
                    sim_require_nnan=True,
                    nc=nc,
                )
            )

        devices = jax.devices()[:n_cores]
        assert len(devices) == n_cores, f"need {n_cores} cores, saw {jax.devices()}"
        self.mesh = Mesh(np.asarray(devices), ("core",))
        self.pspec = PartitionSpec("core")
        in_specs = (self.pspec,) * (n_params + n_outs)
        out_specs = (self.pspec,) * len(out_names)
        self.sharded = jax.jit(
            shard_map(
                _body,
                mesh=self.mesh,
                in_specs=in_specs,
                out_specs=out_specs,
                check_rep=False,
            ),
            keep_unused=True,
        )
        self.n_params = n_params

    def prepare(self, in_maps):
        from jax.sharding import NamedSharding

        per_core = [[np.asarray(m[n]) for n in self.in_names] for m in in_maps]
        concat_in = [
            np.concatenate([per_core[c][i] for c in range(self.n_cores)], axis=0)
            for i in range(self.n_params)
        ]
        concat_zeros = [
            np.zeros((self.n_cores * z.shape[0], *z.shape[1:]), z.dtype)
            for z in self.zero_outs
        ]
        sharding = NamedSharding(self.mesh, self.pspec)
        args = [self.jax.device_put(a, sharding) for a in concat_in + concat_zeros]
        self.jax.block_until_ready(args)
        return args

    def run(self, args):
        out = self.sharded(*args)
        self.jax.block_until_ready(out)
        return out

    def results(self, out_arrs):
        return [
            {
                name: np.asarray(out_arrs[i]).reshape(
                    self.n_cores, *self.out_avals[i].shape
                )[c]
                for i, name in enumerate(self.out_names)
            }
            for c in range(self.n_cores)
        ]


_RUNNER = None


def _get_runner():
    global _RUNNER
    if _RUNNER is None:
        _RUNNER = SpmdRunner(build_nc())
    return _RUNNER


def kernel(x, w_attn, w_o):
    runner = _get_runner()
    in_maps = shard_inputs(x, w_attn, w_o)
    args = runner.prepare(in_maps)
    out = runner.run(args)
    return unshard_output(runner.results(out))
